# revision 9
# baseline (speedup 1.0000x reference)
"""MoE routing kernel for TRN2 (8 NeuronCores).

The reference MoE applies row 0's top-2 expert choice (indices and softmax
weights) to the entire batch, so the whole module collapses to

    out = x @ (w0*We[i0] + w1*We[i1]).T + (w0*be[i0] + w1*be[i1])

a single [16384,2048] @ [2048,2048] matmul with bias. Host does the tiny
row-0 gating and combines the two selected experts; the device runs the
matmul data-parallel over tokens (2048 tokens per core, no collectives).

v3 schedule (trace-driven): inputs are bf16 (x stationary, W moving),
PSUM/bias/out stay fp32. Warm-up matmuls on memset scratch ramp the PE
HAM throttle before the first W chunk lands. All DRAM inputs are
partition-major so each input needs only a handful of dma_starts (15
total) — v2 lost ~8us at phase boundaries to completion-semaphore lane
aliasing between its 49 input triggers and the eviction path. Matmuls
use a [128,1024] moving operand (one LDWEIGHTS + two MATMULs) into
two-bank PSUM tiles. Stage A chases the W stream over m0..3 in two
n-pair phases; stage B runs m4..15 k-inner against resident W.
"""

import os
import sys

import numpy as np

if "/opt/trn_rl_repo" not in sys.path:
    sys.path.insert(0, "/opt/trn_rl_repo")

N, D, E, TOPK = 16384, 2048, 8, 2
N_CORES = 8
M_SHARD = N // N_CORES  # 2048 tokens per core
P = 128
KT = D // P             # 16 contraction slabs
MT = M_SHARD // P       # 16 m tiles
NF = 512
MA = 4                  # m-tiles covered in stage A
NPAIR = 2 * NF          # 1024-wide moving operand / psum tile
KGRP = [(0, 1), (1, 2), (2, 3), (3, 4), (4, 6), (6, 8), (8, 12),
        (12, 16)]  # k-group DMA granularity
WARM_MMS = 26

_CACHE = {}


def _build_nc():
    import concourse.tile as tile
    from concourse import bacc, mybir

    nc = bacc.Bacc(None, target_bir_lowering=False)
    f32 = mybir.dt.float32
    bf16 = mybir.dt.bfloat16

    # Partition-major DRAM layouts: few triggers, long contiguous runs.
    xA = nc.dram_tensor("xA", [P, KT, MA * P], bf16, kind="ExternalInput")
    xB = nc.dram_tensor("xB", [MT - MA, P, KT * P], bf16,
                        kind="ExternalInput")
    w01 = nc.dram_tensor("w01", [P, KT, 2, NF], bf16, kind="ExternalInput")
    w23 = nc.dram_tensor("w23", [P, KT, 2, NF], bf16, kind="ExternalInput")
    bias = nc.dram_tensor("bias", [P, D], f32, kind="ExternalInput")
    out = nc.dram_tensor("out", [M_SHARD, D], f32, kind="ExternalOutput")

    with tile.TileContext(nc) as tc:
        with tc.tile_pool(name="wpool", bufs=1) as wpool, \
             tc.tile_pool(name="xpool", bufs=1) as xpool, \
             tc.tile_pool(name="bpool", bufs=1) as bpool, \
             tc.tile_pool(name="warm", bufs=1) as warm_pool, \
             tc.tile_pool(name="opool", bufs=6) as opool, \
             tc.tile_pool(name="psum", bufs=1, space="PSUM") as psum_pool:

            # Warm-up: ramp the PE power state while the first DMAs fly.
            warm_w = warm_pool.tile([P, P], bf16, name="warm_w", tag="warm_w")
            warm_x = warm_pool.tile([P, P], bf16, name="warm_x", tag="warm_x")
            nc.vector.memset(warm_w[:, :], 0.0)
            nc.vector.memset(warm_x[:, :], 0.0)
            ps_warm = psum_pool.tile([P, NPAIR], f32, name="ps_warm", tag="d0")
            for _ in range(WARM_MMS):
                nc.tensor.matmul(ps_warm[:, :P], lhsT=warm_w[:, :],
                                 rhs=warm_x[:, :], start=True, stop=True)

            # Input DMAs (sync/SP queue) in consumption order, 15 triggers:
            # interleaved w01/xA k-groups, then w23 halves, bias, xB halves.
            w01t, xat = [None] * len(KGRP), [None] * len(KGRP)
            for g, (a, b) in enumerate(KGRP):
                t = wpool.tile([P, (b - a) * 2 * NF], bf16, name=f"w01_{g}",
                               tag=f"w01_{g}")
                if g == 0:
                    # split halves so the first matmul's rhs lands sooner
                    nc.sync.dma_start(out=t[:, :NF], in_=w01[:, 0:1, 0])
                    nc.sync.dma_start(out=t[:, NF:], in_=w01[:, 0:1, 1])
                elif g in (1, 2):
                    # k1/k2 via the idle SWDGE queue: parallel trigger
                    # latency, arrives before the PE finishes k0/k1
                    nc.gpsimd.dma_start(out=t[:, :], in_=w01[:, a:b])
                else:
                    nc.sync.dma_start(out=t[:, :], in_=w01[:, a:b])
                w01t[g] = t
                t = xpool.tile([P, (b - a) * MA * P], bf16, name=f"xa{g}",
                               tag=f"xa{g}")
                nc.scalar.dma_start(out=t[:, :], in_=xA[:, a:b])
                xat[g] = t
            w23t = [None, None]

            def load_w23(h):
                t = wpool.tile([P, 8 * 2 * NF], bf16, name=f"w23_{h}",
                               tag=f"w23_{h}")
                nc.sync.dma_start(out=t[:, :], in_=w23[:, 8 * h:8 * (h + 1)])
                w23t[h] = t

            load_w23(0)
            bias_t = bpool.tile([P, D], f32, name="bias_t", tag="bias_t")
            nc.sync.dma_start(out=bias_t[:, :], in_=bias[:, :])
            load_w23(1)
            # xb tiles rotate 3 tags: trigger N+3 waits until the
            # consumer of trigger N is done, so at most 3 xb DMAs hold
            # completion lanes at once (the eviction path needs lanes too).
            xbt = [None] * (MT - MA)
            for ml in range(MT - MA):
                t = xpool.tile([P, KT * P], bf16, name=f"xb{ml}",
                               tag=f"xb{ml % 3}")
                nc.sync.dma_start(out=t[:, :], in_=xB[ml])
                xbt[ml] = t

            def grp(k):
                for g, (a, b) in enumerate(KGRP):
                    if a <= k < b:
                        return g, k - a
                raise AssertionError

            def xa_sl(k, m):
                g, kl = grp(k)
                o = (kl * MA + m) * P
                return xat[g][:, o:o + P]

            def w_sl(ph, k):
                if ph == 0:
                    g, kl = grp(k)
                    return w01t[g][:, kl * NPAIR:(kl + 1) * NPAIR]
                h, kl = divmod(k, 8)
                return w23t[h][:, kl * NPAIR:(kl + 1) * NPAIR]

            def xb_sl(k, mi):
                return xbt[mi - MA][:, k * P:(k + 1) * P]

            def evict(ps, mi, ph):
                ot = opool.tile([P, NPAIR], f32, name="ot", tag="ot")
                nc.vector.tensor_add(
                    ot[:, :], ps[:, :],
                    bias_t[:, ph * NPAIR:(ph + 1) * NPAIR],
                )
                nc.scalar.dma_start(
                    out=out[mi * P:(mi + 1) * P,
                            ph * NPAIR:(ph + 1) * NPAIR],
                    in_=ot[:, :],
                )

            # Stage A: m0..3, k-outer chasing the W stream, one n-pair
            # phase at a time; 4 two-bank PSUM tiles per phase.
            for ph in range(2):
                pa = [psum_pool.tile([P, NPAIR], f32, name=f"pa{ph}_{m}",
                                     tag=f"d{m}") for m in range(MA)]
                for k in range(KT):
                    w2 = w_sl(ph, k)
                    order = ([(j, m) for j in range(2) for m in range(MA)]
                             if k == 0 else
                             [(j, m) for m in range(MA) for j in range(2)])
                    for j, m in order:
                        nc.tensor.matmul(
                            pa[m][:, j * NF:(j + 1) * NF],
                            lhsT=xa_sl(k, m),
                            rhs=w2[:, j * NF:(j + 1) * NF],
                            start=(k == 0),
                            stop=(k == KT - 1),
                        )
                for m in range(MA):
                    evict(pa[m], m, ph)

            # Stage B: m4..15, k-inner against resident W; each n-pair
            # half computes and evicts sequentially (smoother eviction
            # cadence, short tail after the last matmul).
            cnt = 0
            for mi in range(MA, MT):
                for ph in range(2):
                    ps = psum_pool.tile([P, NPAIR], f32, name="psB",
                                        tag=f"d{cnt % 4}")
                    cnt += 1
                    for k in range(KT):
                        w2 = w_sl(ph, k)
                        for j in range(2):
                            nc.tensor.matmul(
                                ps[:, j * NF:(j + 1) * NF],
                                lhsT=xb_sl(k, mi),
                                rhs=w2[:, j * NF:(j + 1) * NF],
                                start=(k == 0), stop=(k == KT - 1))
                    if mi == MT - 1 and ph == 1:
                        for j in range(2):
                            ot = opool.tile([P, NF], f32, name="otf",
                                            tag=f"otf{j}")
                            nc.vector.tensor_add(
                                ot[:, :], ps[:, j * NF:(j + 1) * NF],
                                bias_t[:, ph * NPAIR + j * NF:
                                       ph * NPAIR + (j + 1) * NF])
                            nc.scalar.dma_start(
                                out=out[mi * P:(mi + 1) * P,
                                        ph * NPAIR + j * NF:
                                        ph * NPAIR + (j + 1) * NF],
                                in_=ot[:, :])
                    else:
                        evict(ps, mi, ph)

    nc.compile()
    return nc


def _get_nc():
    if "nc" not in _CACHE:
        _CACHE["nc"] = _build_nc()
    return _CACHE["nc"]


def _ensure_ntff_hook():
    """Register the axon NTFF profile hook (the image's antenv lacks
    axon_hooks; recreate it and wire the ctypes hook from trn_boot)."""
    import types

    try:
        from antenv.axon_hooks import get_axon_ntff_profile_hook  # noqa: F401
        return
    except ImportError:
        pass
    try:
        import antenv
        from trn_agent_boot.trn_boot import _ntff_profile_via_ctypes

        mod = types.ModuleType("antenv.axon_hooks")
        _state = {"hook": None}
        mod.set_axon_ntff_profile_hook = lambda h: _state.__setitem__("hook", h)
        mod.get_axon_ntff_profile_hook = lambda: _state["hook"]
        sys.modules["antenv.axon_hooks"] = mod
        antenv.axon_hooks = mod
        mod.set_axon_ntff_profile_hook(
            _ntff_profile_via_ctypes("/opt/axon/libaxon_pjrt.so")
        )
        # avoid the S3 artifact upload in the trace path
        import concourse.bass_utils as bu

        bu.upload_artifacts = lambda tmpdir: tmpdir
    except Exception as e:  # profiling is best-effort
        print(f"NTFF hook setup failed: {e}", file=sys.stderr)


def kernel(x, Wg, bg, We, be):
    import ml_dtypes
    from concourse.bass_utils import run_bass_kernel_spmd

    bf16 = ml_dtypes.bfloat16

    x = np.asarray(x, dtype=np.float32)
    Wg = np.asarray(Wg, dtype=np.float32)
    bg = np.asarray(bg, dtype=np.float32)
    We = np.asarray(We, dtype=np.float32)
    be = np.asarray(be, dtype=np.float32)

    # Row-0 gating on host (16K FLOPs): softmax over 8 logits, top-2.
    logits = x[0].astype(np.float64) @ Wg.astype(np.float64).T + bg.astype(
        np.float64
    )
    probs = np.exp(logits - logits.max())
    probs /= probs.sum()
    idx = np.argsort(-probs, kind="stable")[:TOPK]
    w0 = probs[idx]

    Wc = w0[0] * We[idx[0]].astype(np.float64) + w0[1] * We[idx[1]].astype(
        np.float64
    )
    bc = w0[0] * be[idx[0]].astype(np.float64) + w0[1] * be[idx[1]].astype(
        np.float64
    )
    WcT = np.ascontiguousarray(Wc.T)  # [d, o]
    warr = WcT.reshape(KT, P, 4, NF)  # [k, p, n4, f]
    w01_np = np.ascontiguousarray(
        warr[:, :, 0:2, :].transpose(1, 0, 2, 3)).astype(bf16)
    w23_np = np.ascontiguousarray(
        warr[:, :, 2:4, :].transpose(1, 0, 2, 3)).astype(bf16)
    bias = np.ascontiguousarray(
        np.broadcast_to(bc.astype(np.float32), (P, D))
    )

    nc = _get_nc()
    in_maps = []
    for c in range(N_CORES):
        xsh = x[c * M_SHARD:(c + 1) * M_SHARD]           # [m, d]
        xT = np.ascontiguousarray(xsh.T).astype(bf16)    # [d, m]
        x3 = xT.reshape(KT, P, M_SHARD)                  # [k, p, m]
        xa = np.ascontiguousarray(x3[:, :, :MA * P].transpose(1, 0, 2))
        # [k, p, (ml mm)] -> [ml, p, (k mm)]
        xb5 = x3[:, :, MA * P:].reshape(KT, P, MT - MA, P)
        xbm = np.ascontiguousarray(xb5.transpose(2, 1, 0, 3)).reshape(
            MT - MA, P, KT * P)
        in_maps.append({"xA": xa, "xB": xbm,
                        "w01": w01_np, "w23": w23_np, "bias": bias})

    trace = bool(int(os.environ.get("KERNEL_TRACE", "0")))
    tmpdir = None
    if trace:
        import tempfile

        _ensure_ntff_hook()
        tmpdir = tempfile.mkdtemp(prefix="moe_trace_")
        _CACHE["last_tmpdir"] = tmpdir
    res = run_bass_kernel_spmd(
        nc, in_maps, core_ids=list(range(N_CORES)), trace=trace, tmpdir=tmpdir
    )
    _CACHE["last_results"] = res

    return np.concatenate(
        [res.results[c]["out"] for c in range(N_CORES)], axis=0
    )


# revision 10
# speedup vs baseline: 1.0027x; 1.0027x over previous
"""MoE routing kernel for TRN2 (8 NeuronCores).

The reference MoE applies row 0's top-2 expert choice (indices and softmax
weights) to the entire batch, so the whole module collapses to

    out = x @ (w0*We[i0] + w1*We[i1]).T + (w0*be[i0] + w1*be[i1])

a single [16384,2048] @ [2048,2048] matmul with bias. Host does the tiny
row-0 gating and combines the two selected experts; the device runs the
matmul data-parallel over tokens (2048 tokens per core, no collectives).

v3 schedule (trace-driven): inputs are bf16 (x stationary, W moving),
PSUM/bias/out stay fp32. Warm-up matmuls on memset scratch ramp the PE
HAM throttle before the first W chunk lands. All DRAM inputs are
partition-major so each input needs only a handful of dma_starts (15
total) — v2 lost ~8us at phase boundaries to completion-semaphore lane
aliasing between its 49 input triggers and the eviction path. Matmuls
use a [128,1024] moving operand (one LDWEIGHTS + two MATMULs) into
two-bank PSUM tiles. Stage A chases the W stream over m0..3 in two
n-pair phases; stage B runs m4..15 k-inner against resident W.
"""

import os
import sys

import numpy as np

if "/opt/trn_rl_repo" not in sys.path:
    sys.path.insert(0, "/opt/trn_rl_repo")

N, D, E, TOPK = 16384, 2048, 8, 2
N_CORES = 8
M_SHARD = N // N_CORES  # 2048 tokens per core
P = 128
KT = D // P             # 16 contraction slabs
MT = M_SHARD // P       # 16 m tiles
NF = 512
MA = 4                  # m-tiles covered in stage A
NPAIR = 2 * NF          # 1024-wide moving operand / psum tile
KGRP = [(0, 1), (1, 2), (2, 3), (3, 4), (4, 6), (6, 8), (8, 12),
        (12, 16)]  # k-group DMA granularity
WARM_MMS = 28

_CACHE = {}


def _build_nc():
    import concourse.tile as tile
    from concourse import bacc, mybir

    nc = bacc.Bacc(None, target_bir_lowering=False)
    f32 = mybir.dt.float32
    bf16 = mybir.dt.bfloat16

    # Partition-major DRAM layouts: few triggers, long contiguous runs.
    xA = nc.dram_tensor("xA", [P, KT, MA * P], bf16, kind="ExternalInput")
    xB = nc.dram_tensor("xB", [MT - MA, P, KT * P], bf16,
                        kind="ExternalInput")
    w01 = nc.dram_tensor("w01", [P, KT, 2, NF], bf16, kind="ExternalInput")
    w23 = nc.dram_tensor("w23", [P, KT, 2, NF], bf16, kind="ExternalInput")
    bias = nc.dram_tensor("bias", [P, D], f32, kind="ExternalInput")
    out = nc.dram_tensor("out", [M_SHARD, D], f32, kind="ExternalOutput")

    with tile.TileContext(nc) as tc:
        with tc.tile_pool(name="wpool", bufs=1) as wpool, \
             tc.tile_pool(name="xpool", bufs=1) as xpool, \
             tc.tile_pool(name="bpool", bufs=1) as bpool, \
             tc.tile_pool(name="warm", bufs=1) as warm_pool, \
             tc.tile_pool(name="opool", bufs=6) as opool, \
             tc.tile_pool(name="psum", bufs=1, space="PSUM") as psum_pool:

            # Warm-up: ramp the PE power state while the first DMAs fly.
            warm_w = warm_pool.tile([P, P], bf16, name="warm_w", tag="warm_w")
            warm_x = warm_pool.tile([P, P], bf16, name="warm_x", tag="warm_x")
            nc.vector.memset(warm_w[:, :], 0.0)
            nc.vector.memset(warm_x[:, :], 0.0)
            ps_warm = psum_pool.tile([P, NPAIR], f32, name="ps_warm", tag="d0")
            for _ in range(WARM_MMS):
                nc.tensor.matmul(ps_warm[:, :P], lhsT=warm_w[:, :],
                                 rhs=warm_x[:, :], start=True, stop=True)

            # Input DMAs (sync/SP queue) in consumption order, 15 triggers:
            # interleaved w01/xA k-groups, then w23 halves, bias, xB halves.
            w01t, xat = [None] * len(KGRP), [None] * len(KGRP)
            for g, (a, b) in enumerate(KGRP):
                t = wpool.tile([P, (b - a) * 2 * NF], bf16, name=f"w01_{g}",
                               tag=f"w01_{g}")
                nc.sync.dma_start(out=t[:, :], in_=w01[:, a:b])
                w01t[g] = t
                t = xpool.tile([P, (b - a) * MA * P], bf16, name=f"xa{g}",
                               tag=f"xa{g}")
                nc.scalar.dma_start(out=t[:, :], in_=xA[:, a:b])
                xat[g] = t
            w23t = [None, None]

            def load_w23(h):
                t = wpool.tile([P, 8 * 2 * NF], bf16, name=f"w23_{h}",
                               tag=f"w23_{h}")
                nc.sync.dma_start(out=t[:, :], in_=w23[:, 8 * h:8 * (h + 1)])
                w23t[h] = t

            load_w23(0)
            bias_t = bpool.tile([P, D], f32, name="bias_t", tag="bias_t")
            nc.sync.dma_start(out=bias_t[:, :], in_=bias[:, :])
            load_w23(1)
            # xb tiles rotate 3 tags: trigger N+3 waits until the
            # consumer of trigger N is done, so at most 3 xb DMAs hold
            # completion lanes at once (the eviction path needs lanes too).
            xbt = [None] * (MT - MA)
            for ml in range(MT - MA):
                t = xpool.tile([P, KT * P], bf16, name=f"xb{ml}",
                               tag=f"xb{ml % 3}")
                nc.sync.dma_start(out=t[:, :], in_=xB[ml])
                xbt[ml] = t

            def grp(k):
                for g, (a, b) in enumerate(KGRP):
                    if a <= k < b:
                        return g, k - a
                raise AssertionError

            def xa_sl(k, m):
                g, kl = grp(k)
                o = (kl * MA + m) * P
                return xat[g][:, o:o + P]

            def w_sl(ph, k):
                if ph == 0:
                    g, kl = grp(k)
                    return w01t[g][:, kl * NPAIR:(kl + 1) * NPAIR]
                h, kl = divmod(k, 8)
                return w23t[h][:, kl * NPAIR:(kl + 1) * NPAIR]

            def xb_sl(k, mi):
                return xbt[mi - MA][:, k * P:(k + 1) * P]

            def evict(ps, mi, ph):
                ot = opool.tile([P, NPAIR], f32, name="ot", tag="ot")
                nc.vector.tensor_add(
                    ot[:, :], ps[:, :],
                    bias_t[:, ph * NPAIR:(ph + 1) * NPAIR],
                )
                nc.scalar.dma_start(
                    out=out[mi * P:(mi + 1) * P,
                            ph * NPAIR:(ph + 1) * NPAIR],
                    in_=ot[:, :],
                )

            # Stage A: m0..3, k-outer chasing the W stream, one n-pair
            # phase at a time; 4 two-bank PSUM tiles per phase.
            for ph in range(2):
                pa = [psum_pool.tile([P, NPAIR], f32, name=f"pa{ph}_{m}",
                                     tag=f"d{m}") for m in range(MA)]
                for k in range(KT):
                    w2 = w_sl(ph, k)
                    order = ([(j, m) for j in range(2) for m in range(MA)]
                             if k == 0 else
                             [(j, m) for m in range(MA) for j in range(2)])
                    for j, m in order:
                        nc.tensor.matmul(
                            pa[m][:, j * NF:(j + 1) * NF],
                            lhsT=xa_sl(k, m),
                            rhs=w2[:, j * NF:(j + 1) * NF],
                            start=(k == 0),
                            stop=(k == KT - 1),
                        )
                for m in range(MA):
                    evict(pa[m], m, ph)

            # Stage B: m4..15, k-inner against resident W; each n-pair
            # half computes and evicts sequentially (smoother eviction
            # cadence, short tail after the last matmul).
            cnt = 0
            for mi in range(MA, MT):
                for ph in range(2):
                    ps = psum_pool.tile([P, NPAIR], f32, name="psB",
                                        tag=f"d{cnt % 4}")
                    cnt += 1
                    for k in range(KT):
                        w2 = w_sl(ph, k)
                        for j in range(2):
                            nc.tensor.matmul(
                                ps[:, j * NF:(j + 1) * NF],
                                lhsT=xb_sl(k, mi),
                                rhs=w2[:, j * NF:(j + 1) * NF],
                                start=(k == 0), stop=(k == KT - 1))
                    if mi == MT - 1 and ph == 1:
                        for j in range(2):
                            ot = opool.tile([P, NF], f32, name="otf",
                                            tag=f"otf{j}")
                            nc.vector.tensor_add(
                                ot[:, :], ps[:, j * NF:(j + 1) * NF],
                                bias_t[:, ph * NPAIR + j * NF:
                                       ph * NPAIR + (j + 1) * NF])
                            nc.scalar.dma_start(
                                out=out[mi * P:(mi + 1) * P,
                                        ph * NPAIR + j * NF:
                                        ph * NPAIR + (j + 1) * NF],
                                in_=ot[:, :])
                    else:
                        evict(ps, mi, ph)

    nc.compile()
    return nc


def _get_nc():
    if "nc" not in _CACHE:
        _CACHE["nc"] = _build_nc()
    return _CACHE["nc"]


def _ensure_ntff_hook():
    """Register the axon NTFF profile hook (the image's antenv lacks
    axon_hooks; recreate it and wire the ctypes hook from trn_boot)."""
    import types

    try:
        from antenv.axon_hooks import get_axon_ntff_profile_hook  # noqa: F401
        return
    except ImportError:
        pass
    try:
        import antenv
        from trn_agent_boot.trn_boot import _ntff_profile_via_ctypes

        mod = types.ModuleType("antenv.axon_hooks")
        _state = {"hook": None}
        mod.set_axon_ntff_profile_hook = lambda h: _state.__setitem__("hook", h)
        mod.get_axon_ntff_profile_hook = lambda: _state["hook"]
        sys.modules["antenv.axon_hooks"] = mod
        antenv.axon_hooks = mod
        mod.set_axon_ntff_profile_hook(
            _ntff_profile_via_ctypes("/opt/axon/libaxon_pjrt.so")
        )
        # avoid the S3 artifact upload in the trace path
        import concourse.bass_utils as bu

        bu.upload_artifacts = lambda tmpdir: tmpdir
    except Exception as e:  # profiling is best-effort
        print(f"NTFF hook setup failed: {e}", file=sys.stderr)


def kernel(x, Wg, bg, We, be):
    import ml_dtypes
    from concourse.bass_utils import run_bass_kernel_spmd

    bf16 = ml_dtypes.bfloat16

    x = np.asarray(x, dtype=np.float32)
    Wg = np.asarray(Wg, dtype=np.float32)
    bg = np.asarray(bg, dtype=np.float32)
    We = np.asarray(We, dtype=np.float32)
    be = np.asarray(be, dtype=np.float32)

    # Row-0 gating on host (16K FLOPs): softmax over 8 logits, top-2.
    logits = x[0].astype(np.float64) @ Wg.astype(np.float64).T + bg.astype(
        np.float64
    )
    probs = np.exp(logits - logits.max())
    probs /= probs.sum()
    idx = np.argsort(-probs, kind="stable")[:TOPK]
    w0 = probs[idx]

    Wc = w0[0] * We[idx[0]].astype(np.float64) + w0[1] * We[idx[1]].astype(
        np.float64
    )
    bc = w0[0] * be[idx[0]].astype(np.float64) + w0[1] * be[idx[1]].astype(
        np.float64
    )
    WcT = np.ascontiguousarray(Wc.T)  # [d, o]
    warr = WcT.reshape(KT, P, 4, NF)  # [k, p, n4, f]
    w01_np = np.ascontiguousarray(
        warr[:, :, 0:2, :].transpose(1, 0, 2, 3)).astype(bf16)
    w23_np = np.ascontiguousarray(
        warr[:, :, 2:4, :].transpose(1, 0, 2, 3)).astype(bf16)
    bias = np.ascontiguousarray(
        np.broadcast_to(bc.astype(np.float32), (P, D))
    )

    nc = _get_nc()
    in_maps = []
    for c in range(N_CORES):
        xsh = x[c * M_SHARD:(c + 1) * M_SHARD]           # [m, d]
        xT = np.ascontiguousarray(xsh.T).astype(bf16)    # [d, m]
        x3 = xT.reshape(KT, P, M_SHARD)                  # [k, p, m]
        xa = np.ascontiguousarray(x3[:, :, :MA * P].transpose(1, 0, 2))
        # [k, p, (ml mm)] -> [ml, p, (k mm)]
        xb5 = x3[:, :, MA * P:].reshape(KT, P, MT - MA, P)
        xbm = np.ascontiguousarray(xb5.transpose(2, 1, 0, 3)).reshape(
            MT - MA, P, KT * P)
        in_maps.append({"xA": xa, "xB": xbm,
                        "w01": w01_np, "w23": w23_np, "bias": bias})

    trace = bool(int(os.environ.get("KERNEL_TRACE", "0")))
    tmpdir = None
    if trace:
        import tempfile

        _ensure_ntff_hook()
        tmpdir = tempfile.mkdtemp(prefix="moe_trace_")
        _CACHE["last_tmpdir"] = tmpdir
    res = run_bass_kernel_spmd(
        nc, in_maps, core_ids=list(range(N_CORES)), trace=trace, tmpdir=tmpdir
    )
    _CACHE["last_results"] = res

    return np.concatenate(
        [res.results[c]["out"] for c in range(N_CORES)], axis=0
    )


# revision 11
# speedup vs baseline: 1.0112x; 1.0084x over previous
"""MoE routing kernel for TRN2 (8 NeuronCores).

The reference MoE applies row 0's top-2 expert choice (indices and softmax
weights) to the entire batch, so the whole module collapses to

    out = x @ (w0*We[i0] + w1*We[i1]).T + (w0*be[i0] + w1*be[i1])

a single [16384,2048] @ [2048,2048] matmul with bias. Host does the tiny
row-0 gating and combines the two selected experts; the device runs the
matmul data-parallel over tokens (2048 tokens per core, no collectives).

v3 schedule (trace-driven): inputs are bf16 (x stationary, W moving),
PSUM/bias/out stay fp32. Warm-up matmuls on memset scratch ramp the PE
HAM throttle before the first W chunk lands. All DRAM inputs are
partition-major so each input needs only a handful of dma_starts (15
total) — v2 lost ~8us at phase boundaries to completion-semaphore lane
aliasing between its 49 input triggers and the eviction path. Matmuls
use a [128,1024] moving operand (one LDWEIGHTS + two MATMULs) into
two-bank PSUM tiles. Stage A chases the W stream over m0..3 in two
n-pair phases; stage B runs m4..15 k-inner against resident W.
"""

import os
import sys

import numpy as np

if "/opt/trn_rl_repo" not in sys.path:
    sys.path.insert(0, "/opt/trn_rl_repo")

N, D, E, TOPK = 16384, 2048, 8, 2
N_CORES = 8
M_SHARD = N // N_CORES  # 2048 tokens per core
P = 128
KT = D // P             # 16 contraction slabs
MT = M_SHARD // P       # 16 m tiles
NF = 512
MA = 4                  # m-tiles covered in stage A
NPAIR = 2 * NF          # 1024-wide moving operand / psum tile
KGRP = [(0, 1), (1, 2), (2, 3), (3, 4), (4, 6), (6, 8), (8, 12),
        (12, 16)]  # k-group DMA granularity
WARM_MMS = 34

_CACHE = {}


def _build_nc():
    import concourse.tile as tile
    from concourse import bacc, mybir

    nc = bacc.Bacc(None, target_bir_lowering=False)
    f32 = mybir.dt.float32
    bf16 = mybir.dt.bfloat16

    # Partition-major DRAM layouts: few triggers, long contiguous runs.
    xA = nc.dram_tensor("xA", [P, KT, MA * P], bf16, kind="ExternalInput")
    xB = nc.dram_tensor("xB", [MT - MA, P, KT * P], bf16,
                        kind="ExternalInput")
    w01 = nc.dram_tensor("w01", [P, KT, 2, NF], bf16, kind="ExternalInput")
    w23 = nc.dram_tensor("w23", [P, KT, 2, NF], bf16, kind="ExternalInput")
    bias = nc.dram_tensor("bias", [P, D], f32, kind="ExternalInput")
    out = nc.dram_tensor("out", [M_SHARD, D], f32, kind="ExternalOutput")

    with tile.TileContext(nc) as tc:
        with tc.tile_pool(name="wpool", bufs=1) as wpool, \
             tc.tile_pool(name="xpool", bufs=1) as xpool, \
             tc.tile_pool(name="bpool", bufs=1) as bpool, \
             tc.tile_pool(name="warm", bufs=1) as warm_pool, \
             tc.tile_pool(name="opool", bufs=6) as opool, \
             tc.tile_pool(name="psum", bufs=1, space="PSUM") as psum_pool:

            # Warm-up: ramp the PE power state while the first DMAs fly.
            warm_w = warm_pool.tile([P, P], bf16, name="warm_w", tag="warm_w")
            warm_x = warm_pool.tile([P, P], bf16, name="warm_x", tag="warm_x")
            nc.vector.memset(warm_w[:, :], 0.0)
            nc.vector.memset(warm_x[:, :], 0.0)
            ps_warm = psum_pool.tile([P, NPAIR], f32, name="ps_warm", tag="d0")
            for _ in range(WARM_MMS):
                nc.tensor.matmul(ps_warm[:, :P], lhsT=warm_w[:, :],
                                 rhs=warm_x[:, :], start=True, stop=True)

            # Input DMAs (sync/SP queue) in consumption order, 15 triggers:
            # interleaved w01/xA k-groups, then w23 halves, bias, xB halves.
            w01t, xat = [None] * len(KGRP), [None] * len(KGRP)
            for g, (a, b) in enumerate(KGRP):
                t = wpool.tile([P, (b - a) * 2 * NF], bf16, name=f"w01_{g}",
                               tag=f"w01_{g}")
                if g == 0:
                    # split halves so the first matmul's rhs lands sooner
                    nc.sync.dma_start(out=t[:, :NF], in_=w01[:, 0:1, 0])
                    nc.sync.dma_start(out=t[:, NF:], in_=w01[:, 0:1, 1])
                else:
                    nc.sync.dma_start(out=t[:, :], in_=w01[:, a:b])
                w01t[g] = t
                t = xpool.tile([P, (b - a) * MA * P], bf16, name=f"xa{g}",
                               tag=f"xa{g}")
                nc.scalar.dma_start(out=t[:, :], in_=xA[:, a:b])
                xat[g] = t
            w23t = [None, None]

            def load_w23(h):
                t = wpool.tile([P, 8 * 2 * NF], bf16, name=f"w23_{h}",
                               tag=f"w23_{h}")
                nc.sync.dma_start(out=t[:, :], in_=w23[:, 8 * h:8 * (h + 1)])
                w23t[h] = t

            load_w23(0)
            bias_t = bpool.tile([P, D], f32, name="bias_t", tag="bias_t")
            nc.sync.dma_start(out=bias_t[:, :], in_=bias[:, :])
            load_w23(1)
            # xb tiles rotate 3 tags: trigger N+3 waits until the
            # consumer of trigger N is done, so at most 3 xb DMAs hold
            # completion lanes at once (the eviction path needs lanes too).
            xbt = [None] * (MT - MA)
            for ml in range(MT - MA):
                t = xpool.tile([P, KT * P], bf16, name=f"xb{ml}",
                               tag=f"xb{ml % 3}")
                nc.sync.dma_start(out=t[:, :], in_=xB[ml])
                xbt[ml] = t

            def grp(k):
                for g, (a, b) in enumerate(KGRP):
                    if a <= k < b:
                        return g, k - a
                raise AssertionError

            def xa_sl(k, m):
                g, kl = grp(k)
                o = (kl * MA + m) * P
                return xat[g][:, o:o + P]

            def w_sl(ph, k):
                if ph == 0:
                    g, kl = grp(k)
                    return w01t[g][:, kl * NPAIR:(kl + 1) * NPAIR]
                h, kl = divmod(k, 8)
                return w23t[h][:, kl * NPAIR:(kl + 1) * NPAIR]

            def xb_sl(k, mi):
                return xbt[mi - MA][:, k * P:(k + 1) * P]

            def evict(ps, mi, ph):
                ot = opool.tile([P, NPAIR], f32, name="ot", tag="ot")
                nc.vector.tensor_add(
                    ot[:, :], ps[:, :],
                    bias_t[:, ph * NPAIR:(ph + 1) * NPAIR],
                )
                nc.scalar.dma_start(
                    out=out[mi * P:(mi + 1) * P,
                            ph * NPAIR:(ph + 1) * NPAIR],
                    in_=ot[:, :],
                )

            # Stage A: m0..3, k-outer chasing the W stream, one n-pair
            # phase at a time; 4 two-bank PSUM tiles per phase.
            for ph in range(2):
                pa = [psum_pool.tile([P, NPAIR], f32, name=f"pa{ph}_{m}",
                                     tag=f"d{m}") for m in range(MA)]
                for k in range(KT):
                    w2 = w_sl(ph, k)
                    order = ([(j, m) for j in range(2) for m in range(MA)]
                             if k == 0 else
                             [(j, m) for m in range(MA) for j in range(2)])
                    for j, m in order:
                        nc.tensor.matmul(
                            pa[m][:, j * NF:(j + 1) * NF],
                            lhsT=xa_sl(k, m),
                            rhs=w2[:, j * NF:(j + 1) * NF],
                            start=(k == 0),
                            stop=(k == KT - 1),
                        )
                for m in range(MA):
                    evict(pa[m], m, ph)

            # Stage B: m4..15, k-inner against resident W; each n-pair
            # half computes and evicts sequentially (smoother eviction
            # cadence, short tail after the last matmul).
            cnt = 0
            for mi in range(MA, MT):
                for ph in range(2):
                    ps = psum_pool.tile([P, NPAIR], f32, name="psB",
                                        tag=f"d{cnt % 4}")
                    cnt += 1
                    for k in range(KT):
                        w2 = w_sl(ph, k)
                        for j in range(2):
                            nc.tensor.matmul(
                                ps[:, j * NF:(j + 1) * NF],
                                lhsT=xb_sl(k, mi),
                                rhs=w2[:, j * NF:(j + 1) * NF],
                                start=(k == 0), stop=(k == KT - 1))
                    if mi == MT - 1 and ph == 1:
                        for j in range(2):
                            ot = opool.tile([P, NF], f32, name="otf",
                                            tag=f"otf{j}")
                            nc.vector.tensor_add(
                                ot[:, :], ps[:, j * NF:(j + 1) * NF],
                                bias_t[:, ph * NPAIR + j * NF:
                                       ph * NPAIR + (j + 1) * NF])
                            nc.scalar.dma_start(
                                out=out[mi * P:(mi + 1) * P,
                                        ph * NPAIR + j * NF:
                                        ph * NPAIR + (j + 1) * NF],
                                in_=ot[:, :])
                    else:
                        evict(ps, mi, ph)

    nc.compile()
    return nc


def _get_nc():
    if "nc" not in _CACHE:
        _CACHE["nc"] = _build_nc()
    return _CACHE["nc"]


def _ensure_ntff_hook():
    """Register the axon NTFF profile hook (the image's antenv lacks
    axon_hooks; recreate it and wire the ctypes hook from trn_boot)."""
    import types

    try:
        from antenv.axon_hooks import get_axon_ntff_profile_hook  # noqa: F401
        return
    except ImportError:
        pass
    try:
        import antenv
        from trn_agent_boot.trn_boot import _ntff_profile_via_ctypes

        mod = types.ModuleType("antenv.axon_hooks")
        _state = {"hook": None}
        mod.set_axon_ntff_profile_hook = lambda h: _state.__setitem__("hook", h)
        mod.get_axon_ntff_profile_hook = lambda: _state["hook"]
        sys.modules["antenv.axon_hooks"] = mod
        antenv.axon_hooks = mod
        mod.set_axon_ntff_profile_hook(
            _ntff_profile_via_ctypes("/opt/axon/libaxon_pjrt.so")
        )
        # avoid the S3 artifact upload in the trace path
        import concourse.bass_utils as bu

        bu.upload_artifacts = lambda tmpdir: tmpdir
    except Exception as e:  # profiling is best-effort
        print(f"NTFF hook setup failed: {e}", file=sys.stderr)


def kernel(x, Wg, bg, We, be):
    import ml_dtypes
    from concourse.bass_utils import run_bass_kernel_spmd

    bf16 = ml_dtypes.bfloat16

    x = np.asarray(x, dtype=np.float32)
    Wg = np.asarray(Wg, dtype=np.float32)
    bg = np.asarray(bg, dtype=np.float32)
    We = np.asarray(We, dtype=np.float32)
    be = np.asarray(be, dtype=np.float32)

    # Row-0 gating on host (16K FLOPs): softmax over 8 logits, top-2.
    logits = x[0].astype(np.float64) @ Wg.astype(np.float64).T + bg.astype(
        np.float64
    )
    probs = np.exp(logits - logits.max())
    probs /= probs.sum()
    idx = np.argsort(-probs, kind="stable")[:TOPK]
    w0 = probs[idx]

    Wc = w0[0] * We[idx[0]].astype(np.float64) + w0[1] * We[idx[1]].astype(
        np.float64
    )
    bc = w0[0] * be[idx[0]].astype(np.float64) + w0[1] * be[idx[1]].astype(
        np.float64
    )
    WcT = np.ascontiguousarray(Wc.T)  # [d, o]
    warr = WcT.reshape(KT, P, 4, NF)  # [k, p, n4, f]
    w01_np = np.ascontiguousarray(
        warr[:, :, 0:2, :].transpose(1, 0, 2, 3)).astype(bf16)
    w23_np = np.ascontiguousarray(
        warr[:, :, 2:4, :].transpose(1, 0, 2, 3)).astype(bf16)
    bias = np.ascontiguousarray(
        np.broadcast_to(bc.astype(np.float32), (P, D))
    )

    nc = _get_nc()
    in_maps = []
    for c in range(N_CORES):
        xsh = x[c * M_SHARD:(c + 1) * M_SHARD]           # [m, d]
        xT = np.ascontiguousarray(xsh.T).astype(bf16)    # [d, m]
        x3 = xT.reshape(KT, P, M_SHARD)                  # [k, p, m]
        xa = np.ascontiguousarray(x3[:, :, :MA * P].transpose(1, 0, 2))
        # [k, p, (ml mm)] -> [ml, p, (k mm)]
        xb5 = x3[:, :, MA * P:].reshape(KT, P, MT - MA, P)
        xbm = np.ascontiguousarray(xb5.transpose(2, 1, 0, 3)).reshape(
            MT - MA, P, KT * P)
        in_maps.append({"xA": xa, "xB": xbm,
                        "w01": w01_np, "w23": w23_np, "bias": bias})

    trace = bool(int(os.environ.get("KERNEL_TRACE", "0")))
    tmpdir = None
    if trace:
        import tempfile

        _ensure_ntff_hook()
        tmpdir = tempfile.mkdtemp(prefix="moe_trace_")
        _CACHE["last_tmpdir"] = tmpdir
    res = run_bass_kernel_spmd(
        nc, in_maps, core_ids=list(range(N_CORES)), trace=trace, tmpdir=tmpdir
    )
    _CACHE["last_results"] = res

    return np.concatenate(
        [res.results[c]["out"] for c in range(N_CORES)], axis=0
    )
